# revision 9
# baseline (speedup 1.0000x reference)
"""Multi-head attention (batch=2, seq=2048, d_model=2048, 16 heads, causal)
on 8 Trainium2 NeuronCores.

Sharding (Megatron-style tensor parallel + data parallel):
  core c -> batch b = c // 4, feature block j = c % 4 (4 heads = 512 features).
  Each core computes Q/K/V projections for its 512 feature columns
  (w_q/w_k/w_v column-sliced), attention for its 4 heads, and a partial
  output projection (w_o row-sliced).  The 4 partial outputs per batch
  element are summed on the host (the Megatron row-parallel AllReduce).

Device math (per core), all matmuls in bf16 with fp32 PSUM accumulation
(bf16 measured faster than fp16 on silicon: PE streams bf16 at 1 col/cycle;
fp16 measured ~1.4x slower despite the cost model claiming parity):
  xT  = x[b].T                          [2048 dm, 2048 s]   (host-prepped)
  Q^T = wq_c.T @ ... -> lhsT=wq chunks  [512 f, 2048 s]
  K^T                                    [512 f, 2048 s]
  V   = x @ wv_c                         [2048 s, 512 f]    (+ ones column)
  per head h, per key block kc:  S^T[k, q] = K^T_h[:,kc].T @ Q^T_h
  T = exp(S^T / sqrt(128))  (unnormalized softmax; scores are O(5) so no
      max-subtraction is needed in fp32), causal-masked
  per query block qb: O[q, d|sum] = sum_kc T_kc[:, qb].T @ [V_kc | 1]
  O /= sum  -> transpose via PE -> O^T [512 f, 2048 s]
  out partial = O^T.T @ wo_c             [2048 s, 2048 dmo]  bf16 stores

Pipelining notes (from TimelineSim analysis):
  - persist/stg SBUF pools are hoisted out of the iteration loop.  Per-iter
    pool-open barriers wait for ALL users of the reused address range (for
    persist that's the previous iteration's LAST phase-C matmul) and they
    block the SP sequencer, which stalls the next iteration's weight/xt
    DMA issues -> a ~14us PE bubble at every iteration boundary.
  - all input DMAs are issued BEFORE the phase-A PSUM pools open, because
    the psq pool-open barrier also waits for the previous iteration's
    phase-C psum users.
  - the wo load is issued LAST: its WAR dep (prev iter's phase-C wo reads)
    clears last, and the SP HWDGE queue is in-order.
  - output stores go on the ACT HWDGE queue so next-iter loads on the SP
    queue bypass them; stores are bf16 (error budget has plenty of room).
  - phase B interleaves pass1(h+1) (N=512 score matmuls) into pass2(h)
    (N=129 PV matmuls): the PE reorder window hides the T-block weight
    loads of PV under the 512-col score streams.
"""

import math
import threading
from contextlib import ExitStack

import ml_dtypes
import numpy as np

import concourse.bass as bass
import concourse.mybir as mybir
import concourse.tile as tile
from concourse import bacc
from concourse.masks import make_identity

import os
_DT = os.environ.get("MHA_DTYPE", "bf16")
BF16 = mybir.dt.float16 if _DT == "fp16" else mybir.dt.bfloat16
F32 = mybir.dt.float32
NPBF16 = np.float16 if _DT == "fp16" else ml_dtypes.bfloat16

SEQ = 2048
DM = 2048
HEADS_PER_CORE = 4
F = 512  # features per core
P = 128
NKC = SEQ // P  # 16 key blocks
NR = DM // P  # 16 contraction chunks
SCALE = 1.0 / math.sqrt(128.0)

# compact T-buffer offsets: block kc covers q in [kc*128, 2048)
T_WIDTHS = [SEQ - kc * P for kc in range(NKC)]
T_OFFS = list(np.cumsum([0] + T_WIDTHS[:-1]))
T_TOTAL = int(np.sum(T_WIDTHS))  # 17408


def build_nc(iters: int = 1) -> bacc.Bacc:
    nc = bacc.Bacc("TRN2", num_devices=8)

    xt_h = nc.dram_tensor("xt", [DM, SEQ], BF16, kind="ExternalInput")
    wq_h = nc.dram_tensor("wq", [DM, F], BF16, kind="ExternalInput")
    wk_h = nc.dram_tensor("wk", [DM, F], BF16, kind="ExternalInput")
    wv_h = nc.dram_tensor("wv", [DM, F], BF16, kind="ExternalInput")
    wo_h = nc.dram_tensor("wo", [F, DM], BF16, kind="ExternalInput")
    tri_h = nc.dram_tensor("tri", [P, P], BF16, kind="ExternalInput")
    out_h = nc.dram_tensor("out", [SEQ, DM], BF16, kind="ExternalOutput")

    xt = xt_h.ap()
    wo_r = wo_h.ap().rearrange("(c p) n -> p c n", p=P)  # [128, 4, 2048]
    out_ap = out_h.ap()

    with tile.TileContext(nc) as tc, ExitStack() as octx:
        consts = octx.enter_context(tc.tile_pool(name="consts", bufs=1))
        ident = consts.tile([P, P], BF16)
        make_identity(nc, ident)
        tri_sb = consts.tile([P, P], BF16)
        nc.sync.dma_start(out=tri_sb, in_=tri_h.ap())

        # hoisted across iterations: same-tag tiles reuse the same SBUF slot
        # with fine-grained WAR deps instead of pool-open barriers
        persist = octx.enter_context(tc.tile_pool(name="persist", bufs=1))
        stg = octx.enter_context(tc.tile_pool(name="stg", bufs=3))

        for _it in range(iters):
            qt_sb = persist.tile([P, HEADS_PER_CORE, SEQ], BF16, tag="qt", name="qt_sb")
            kt_sb = persist.tile([P, HEADS_PER_CORE, SEQ], BF16, tag="kt", name="kt_sb")
            v_sb = persist.tile(
                [P, NKC, HEADS_PER_CORE, P + 1], BF16, tag="v", name="v_sb"
            )
            ot_sb = persist.tile([P, HEADS_PER_CORE, SEQ], BF16, tag="ot", name="ot_sb")
            wo_sb = persist.tile(
                [P, HEADS_PER_CORE, DM], BF16, tag="wo", name="wo_sb"
            )

            # ones column for the fused softmax-denominator trick
            nc.vector.memset(v_sb[:, :, :, P : P + 1], 1.0)

            def pass1_chunks(h, t_h, psum_pool, ptag):
                # chunk emitters for T = causal_mask(exp(S^T/sqrt(d)))
                chunks = []
                for kc in range(NKC):
                    w = T_WIDTHS[kc]
                    for c in range((w + 1023) // 1024):
                        def emit(kc=kc, c=c, w=w):
                            off = T_OFFS[kc]
                            q0 = kc * P
                            lhsT = kt_sb[:, h, kc * P : (kc + 1) * P]
                            wc = min(1024, w - c * 1024)
                            ps = psum_pool.tile(
                                [P, 1024], F32, tag=ptag, name="ps1"
                            )
                            for n in range((wc + 511) // 512):
                                nw = min(512, wc - n * 512)
                                o0 = c * 1024 + n * 512
                                nc.tensor.matmul(
                                    ps[:, n * 512 : n * 512 + nw],
                                    lhsT,
                                    qt_sb[:, h, q0 + o0 : q0 + o0 + nw],
                                    start=True,
                                    stop=True,
                                )
                            nc.scalar.activation(
                                t_h[:, off + c * 1024 : off + c * 1024 + wc],
                                ps[:, 0:wc],
                                mybir.ActivationFunctionType.Exp,
                                scale=SCALE,
                            )
                            if c == 0:
                                nc.vector.tensor_mul(
                                    t_h[:, off : off + P],
                                    t_h[:, off : off + P],
                                    tri_sb,
                                )
                        chunks.append(emit)
                return chunks

            # ---------------- Phase A: projections ----------------
            with ExitStack() as actx:
                pa = actx.enter_context(tc.tile_pool(name="pa", bufs=1))
                wpool = actx.enter_context(tc.tile_pool(name="wpool", bufs=2))

                # all loads issued before any PSUM pool opens (pool-open
                # barriers block the SP sequencer on prev-iter psum users)
                w_tiles = {}
                w_rs = {}
                for nm, w_h in (("wq", wq_h), ("wk", wk_h), ("wv", wv_h)):
                    w_rs[nm] = w_h.ap().rearrange("(r p) f -> p r f", p=P)
                    w_tiles[nm] = wpool.tile(
                        [P, NR, F], BF16, tag="w", name=nm
                    )
                # wq lands first so the first matmul isn't queued behind
                # the 8.4MB xt bulk transfer
                nc.sync.dma_start(
                    out=w_tiles["wq"][:, 0:1, :], in_=w_rs["wq"][:, 0:1, :]
                )
                nc.sync.dma_start(
                    out=w_tiles["wq"][:, 1:4, :], in_=w_rs["wq"][:, 1:4, :]
                )
                for rr in range(1, 4):
                    nc.sync.dma_start(
                        out=w_tiles["wq"][:, rr * 4 : (rr + 1) * 4, :],
                        in_=w_rs["wq"][:, rr * 4 : (rr + 1) * 4, :],
                    )
                xt_sb = pa.tile([P, NR, SEQ], BF16, tag="xt", name="xt_sb")
                nc.sync.dma_start(out=xt_sb[:, 0, 0:1024], in_=xt[0:P, 0:1024])
                nc.sync.dma_start(
                    out=xt_sb[:, 0, 1024:2048], in_=xt[0:P, 1024:2048]
                )
                for r in range(1, NR):
                    nc.sync.dma_start(
                        out=xt_sb[:, r, :], in_=xt[r * P : (r + 1) * P, :]
                    )
                for nm in ("wk", "wv"):
                    for rr in range(4):
                        nc.sync.dma_start(
                            out=w_tiles[nm][:, rr * 4 : (rr + 1) * 4, :],
                            in_=w_rs[nm][:, rr * 4 : (rr + 1) * 4, :],
                        )
                # wo last: its WAR dep (prev iter phase-C reads) clears last
                nc.sync.dma_start(out=wo_sb, in_=wo_r)

                psq = actx.enter_context(
                    tc.tile_pool(name="psq", bufs=3, space="PSUM")
                )
                psv = actx.enter_context(
                    tc.tile_pool(name="psv", bufs=2, space="PSUM")
                )

                # Q^T and K^T: psum[f_rel, s] = sum_r w[r,f].T @ xT[r, s]
                for nm, dst in (("wq", qt_sb), ("wk", kt_sb)):
                    w_t = w_tiles[nm]
                    for f in range(HEADS_PER_CORE):
                        pq0 = psq.tile([P, 1024], F32, tag="psq")
                        pq1 = psq.tile([P, 1024], F32, tag="psq")
                        for r in range(NR):
                            lhsT = w_t[:, r, f * P : (f + 1) * P]
                            for half, pq in ((0, pq0), (1, pq1)):
                                for sn in range(2):
                                    s0 = half * 1024 + sn * 512
                                    nc.tensor.matmul(
                                        pq[:, sn * 512 : (sn + 1) * 512],
                                        lhsT,
                                        xt_sb[:, r, s0 : s0 + 512],
                                        start=(r == 0),
                                        stop=(r == NR - 1),
                                    )
                        nc.vector.tensor_copy(dst[:, f, 0:1024], pq0)
                        nc.scalar.copy(dst[:, f, 1024:2048], pq1)

                # V: psum[s_rel, f] = sum_r xT[r, s].T @ wv[r, f]
                wv_t = w_tiles["wv"]
                for sm in range(NKC):
                    pv = psv.tile([P, F], F32, tag="psv")
                    for r in range(NR):
                        nc.tensor.matmul(
                            pv,
                            xt_sb[:, r, sm * P : (sm + 1) * P],
                            wv_t[:, r, :],
                            start=(r == 0),
                            stop=(r == NR - 1),
                        )
                    nc.vector.tensor_copy(
                        v_sb[:, sm, :, 0:P],
                        pv.rearrange("p (h d) -> p h d", h=HEADS_PER_CORE),
                    )

            # ---------------- Phase B: attention ----------------
            # pass2(h) (N=129 PV matmuls, LDW-bound) is interleaved with
            # pass1(h+1) (N=512 score matmuls) so the PE always has big
            # streams in flight while T-block weights load.
            with ExitStack() as bctx:
                pb = bctx.enter_context(tc.tile_pool(name="pb", bufs=2))
                pbo = bctx.enter_context(tc.tile_pool(name="pbo", bufs=3))
                pss = bctx.enter_context(
                    tc.tile_pool(name="pss", bufs=2, space="PSUM")
                )
                pso = bctx.enter_context(
                    tc.tile_pool(name="pso", bufs=2, space="PSUM")
                )
                pst = bctx.enter_context(
                    tc.tile_pool(name="pst", bufs=2, space="PSUM")
                )

                t_cur = pb.tile([P, T_TOTAL], BF16, tag="T", name="t_sb")
                for e in pass1_chunks(0, t_cur, pss, "pss"):
                    e()

                for h in range(HEADS_PER_CORE):
                    t_h = t_cur
                    nxt = []
                    if h + 1 < HEADS_PER_CORE:
                        t_cur = pb.tile([P, T_TOTAL], BF16, tag="T", name="t_sb")
                        nxt = pass1_chunks(h + 1, t_cur, pss, "pss")
                    emitted = 0

                    # pass 2: O accumulation + normalize + transpose
                    for qb in range(NKC):
                        po = pso.tile([P, P + 1], F32, tag="pso")
                        for kc in range(qb + 1):
                            col = T_OFFS[kc] + (qb - kc) * P
                            nc.tensor.matmul(
                                po,
                                t_h[:, col : col + P],
                                v_sb[:, kc, h, :],
                                start=(kc == 0),
                                stop=(kc == qb),
                            )
                        recip = pbo.tile([P, 1], F32, tag="recip")
                        nc.vector.reciprocal(recip, po[:, P : P + 1])
                        o_sb = pbo.tile([P, P], BF16, tag="o")
                        nc.vector.tensor_scalar_mul(o_sb, po[:, 0:P], recip)
                        i4 = qb % 4
                        if i4 == 0:
                            pt = pst.tile([P, 512], BF16, tag="pst")
                        nc.tensor.transpose(
                            pt[:, i4 * P : (i4 + 1) * P], o_sb, ident
                        )
                        if i4 == 3:
                            g = qb // 4
                            nc.vector.tensor_copy(
                                ot_sb[:, h, g * 512 : (g + 1) * 512], pt
                            )
                        want = (qb + 1) * len(nxt) // NKC
                        while emitted < want:
                            nxt[emitted]()
                            emitted += 1

            # ---------------- Phase C: output projection ----------------
            with ExitStack() as cctx:
                pco = cctx.enter_context(
                    tc.tile_pool(name="pco", bufs=2, space="PSUM")
                )
                for sm in range(NKC):
                    po = pco.tile([P, DM], F32, tag="pco")
                    for f in range(HEADS_PER_CORE):
                        lhsT = ot_sb[:, f, sm * P : (sm + 1) * P]
                        for nd in range(4):
                            nc.tensor.matmul(
                                po[:, nd * 512 : (nd + 1) * 512],
                                lhsT,
                                wo_sb[:, f, nd * 512 : (nd + 1) * 512],
                                start=(f == 0),
                                stop=(f == HEADS_PER_CORE - 1),
                            )
                    stage = stg.tile([P, DM], BF16, tag="stage", name="stage")
                    if sm % 2 == 0:
                        nc.vector.tensor_copy(stage, po)
                    else:
                        nc.scalar.copy(stage, po)
                    # store on the ACT HWDGE queue: keeps the in-order SP
                    # queue free for the next iteration's weight/xt loads
                    nc.scalar.dma_start(
                        out=out_ap[sm * P : (sm + 1) * P, :], in_=stage
                    )

    nc.compile()
    return nc


def prep_in_maps(x, mask, w_q, w_k, w_v, w_o):
    """Host-side sharding: per-core input dicts (8 cores)."""
    x = np.asarray(x, dtype=np.float32)
    mask = np.asarray(mask, dtype=np.float32)
    w_q = np.asarray(w_q, dtype=np.float32)
    w_k = np.asarray(w_k, dtype=np.float32)
    w_v = np.asarray(w_v, dtype=np.float32)
    w_o = np.asarray(w_o, dtype=np.float32)

    # tri[k, q] = 1 where allowed (k <= q), from the mask's diagonal block
    tri = np.ascontiguousarray(
        (mask[:P, :P].T == 0.0).astype(NPBF16)
    )
    xts = [np.ascontiguousarray(x[b].T).astype(NPBF16) for b in range(2)]
    in_maps = []
    for c in range(8):
        b, j = divmod(c, 4)
        sl = slice(j * F, (j + 1) * F)
        in_maps.append(
            {
                "xt": xts[b],
                "wq": np.ascontiguousarray(w_q[:, sl]).astype(NPBF16),
                "wk": np.ascontiguousarray(w_k[:, sl]).astype(NPBF16),
                "wv": np.ascontiguousarray(w_v[:, sl]).astype(NPBF16),
                "wo": np.ascontiguousarray(w_o[sl, :]).astype(NPBF16),
                "tri": tri,
            }
        )
    return in_maps


def gather(results):
    """Sum the 4 partial outputs per batch element (bf16 partials, fp32 sum)."""
    out = np.zeros((2, SEQ, DM), np.float32)
    for c in range(8):
        out[c // 4] += np.asarray(results[c]["out"]).astype(np.float32)
    return out


_cache = threading.local()


def kernel(x, mask, w_q, w_k, w_v, w_o):
    from concourse.bass_utils import run_bass_kernel_spmd

    nc = getattr(_cache, "nc", None)
    if nc is None:
        nc = build_nc(1)
        _cache.nc = nc
    in_maps = prep_in_maps(x, mask, w_q, w_k, w_v, w_o)
    res = run_bass_kernel_spmd(nc, in_maps, core_ids=list(range(8)))
    return gather(res.results)
